# revision 25
# baseline (speedup 1.0000x reference)
"""Causal self-attention kernel for 8 trn2 NeuronCores.

Sharding: 4 batches x 2 head-groups (8 heads each). Core c handles
batch c//2, heads (c%2)*8 .. (c%2)*8+8. Each core computes qkv for its
head-group, causal attention, and a partial projection; the host sums
the two head-group partials per batch and adds b_proj.

Device math (per core, all matmuls fp32r):
  qT[c,t]  = sum_k Wq_aug[k,c] xT_aug[k,t]   (bias folded via ones row)
  kT, v    similarly (v in natural [token, col] layout, with an
            interleaved ones column per head for the softmax denominator)
  scoresT[j,i] = kT.T q / 8   per head, exp'd on ScalarE (no max
            subtraction needed: scores ~ N(0,1)), causal-masked by
            multiplying precomputed 0/1 masks on the diagonal blocks
  yT_un[d,i], Z[i] = PV matmul with ones column (m=65)
  yT = yT_un * (1/Z) broadcast (gpsimd partition_broadcast), off the
            PE critical path so PSUM slots recycle fast (keeps HAM warm)
  outT partial = yT.T @ Wp   (natural [token, col] output)
"""

import sys
import os

for _p in ("/opt/trn_rl_repo", "/root/.axon_site/_ro/trn_rl_repo"):
    if os.path.isdir(_p) and _p not in sys.path:
        sys.path.insert(0, _p)

import numpy as np
import concourse.bass as bass  # noqa: F401
import concourse.mybir as mybir
import concourse.tile as tile
from concourse import bacc, bass_utils

F32 = mybir.dt.float32
F32R = mybir.dt.float32r
ActF = mybir.ActivationFunctionType

B, S, D, H = 4, 2048, 1024, 16
NH = 8          # heads per core
HPAIRS = NH // 2
KT = D // 128   # 8 k-tiles over D
N_CORES = 8

_nc_cache = {}


def build_nc(S_tok=S, n_cores=N_CORES):
    key = (S_tok, n_cores)
    if key in _nc_cache:
        return _nc_cache[key]
    IC = S_tok // 512      # query chunks
    NT = S_tok // 128      # token tiles
    nc = bacc.Bacc("TRN2", target_bir_lowering=False, debug=False,
                   num_devices=n_cores)
    xT = nc.dram_tensor("xT", [D, S_tok], F32, kind="ExternalInput").ap()
    Wq = nc.dram_tensor("Wq", [D + 1, 512], F32, kind="ExternalInput").ap()
    Wk = nc.dram_tensor("Wk", [D + 1, 512], F32, kind="ExternalInput").ap()
    Wv = nc.dram_tensor("Wv", [D + 1, 512], F32, kind="ExternalInput").ap()
    Wp = nc.dram_tensor("Wp", [512, D], F32, kind="ExternalInput").ap()
    out = nc.dram_tensor("out", [S_tok, D], F32, kind="ExternalOutput").ap()

    with tile.TileContext(nc) as tc:
        with tc.tile_pool(name="persist", bufs=1) as pp:
            # resident xT (f32r views of the f32 dram rows); DMAs are
            # issued inside the V-pass scope, interleaved with the Wv
            # loads and column-chunked so the first matmuls start early
            xtr = [pp.tile([128, S_tok], F32R, name=f"xtr{k}")
                   for k in range(KT)]
            # constant ones row: rhs/lhsT for the folded-bias K=1 matmuls
            ones_f32 = pp.tile([1, 512], F32, name="ones_f32")
            nc.gpsimd.memset(ones_f32, 1.0)
            ones_row = pp.tile([1, 512], F32R, name="ones_row")
            nc.vector.tensor_copy(ones_row, ones_f32)

            # v in natural layout, 65-stride per head (64 v cols + ones col)
            v_sb = [pp.tile([128, 8 * 65], F32R, name=f"vsb{t}")
                    for t in range(NT)]
            # yT accumulation per head pair [local d, tokens]
            yT = [pp.tile([128, S_tok], F32R, name=f"ytr{h}")
                  for h in range(HPAIRS)]
            # causal masks for the 4 diagonal offsets r = 128*t:
            # mask[p, y] = 1 if y >= p + r else 0
            masks = []
            BF16 = mybir.dt.bfloat16
            for t in range(4):
                wm = 128 * (t + 1)
                m = pp.tile([128, wm], BF16, name=f"mask{t}")
                nc.gpsimd.memset(m, 1.0)
                nc.gpsimd.affine_select(
                    out=m, in_=m, compare_op=mybir.AluOpType.is_ge,
                    fill=0.0, base=-128 * t, pattern=[[1, wm]],
                    channel_multiplier=-1)
                masks.append(m)
            ones8 = pp.tile([128, 8, 1], F32, name="ones8")
            nc.gpsimd.memset(ones8, 1.0)

            # ---- V pass ----
            with tc.tile_pool(name="wv", bufs=1) as wvp, \
                 tc.tile_pool(name="ps1", bufs=4, space="PSUM") as ps1:
                wv = [wvp.tile([128, 512], F32R, name=f"wv{k}")
                      for k in range(KT)]
                wv9 = wvp.tile([1, 512], F32R, name="wv9")
                nc.sync.dma_start(wv9, Wv[D:D + 1, :].bitcast(F32R))
                for k in range(KT):
                    nc.sync.dma_start(
                        wv[k], Wv[k * 128:(k + 1) * 128, :].bitcast(F32R))
                    nc.sync.dma_start(
                        xtr[k][:, 0:512],
                        xT[k * 128:(k + 1) * 128, 0:512].bitcast(F32R))
                for c in range(1, S_tok // 512):
                    cs = slice(c * 512, (c + 1) * 512)
                    for k in range(KT):
                        nc.sync.dma_start(
                            xtr[k][:, cs],
                            xT[k * 128:(k + 1) * 128, cs].bitcast(F32R))
                for g in range(NT // 4):
                    psvs = [ps1.tile([128, 512], F32, tag="psv", name="psv")
                            for _ in range(4)]
                    # k-outer so each wv[k] weight load serves 4 matmuls
                    for k in range(KT):
                        for j in range(4):
                            t = g * 4 + j
                            nc.tensor.matmul(
                                psvs[j], xtr[k][:, t * 128:(t + 1) * 128],
                                wv[k], start=(k == 0), stop=False)
                    for j in range(4):
                        nc.tensor.matmul(psvs[j], ones_row[0:1, 0:128],
                                         wv9, start=False, stop=True)
                    for j in range(4):
                        t = g * 4 + j
                        vv = v_sb[t].rearrange("p (h c) -> p h c", c=65)
                        nc.vector.tensor_copy(
                            vv[:, :, 0:64],
                            psvs[j].rearrange("p (h c) -> p h c", c=64))
                        nc.vector.tensor_copy(vv[:, :, 64:65], ones8)

            # ---- per head-pair: q/k then attention ----
            with tc.tile_pool(name="hsb", bufs=1) as hsb, \
                 tc.tile_pool(name="pss", bufs=2, space="PSUM") as pssp, \
                 tc.tile_pool(name="psy", bufs=4, space="PSUM") as psyp:
                def fetch_w(hp):
                    wq, wk = [], []
                    for k in range(KT):
                        tq = hsb.tile([128, 128], F32R, tag=f"wq{k}",
                                      name="wq")
                        nc.sync.dma_start(
                            tq, Wq[k * 128:(k + 1) * 128,
                                   hp * 128:(hp + 1) * 128].bitcast(F32R))
                        wq.append(tq)
                        tk = hsb.tile([128, 128], F32R, tag=f"wk{k}",
                                      name="wk")
                        nc.sync.dma_start(
                            tk, Wk[k * 128:(k + 1) * 128,
                                   hp * 128:(hp + 1) * 128].bitcast(F32R))
                        wk.append(tk)
                    wq9 = hsb.tile([1, 128], F32R, tag="wq9", name="wq9")
                    nc.sync.dma_start(
                        wq9, Wq[D:D + 1,
                                hp * 128:(hp + 1) * 128].bitcast(F32R))
                    wk9 = hsb.tile([1, 128], F32R, tag="wk9", name="wk9")
                    nc.sync.dma_start(
                        wk9, Wk[D:D + 1,
                                hp * 128:(hp + 1) * 128].bitcast(F32R))
                    return wq, wk, wq9, wk9

                wnext = fetch_w(0)
                for hp in range(HPAIRS):
                    wq, wk, wq9, wk9 = wnext

                    qt = hsb.tile([128, S_tok], F32R, tag="qt", name="qt")
                    kt_t = hsb.tile([128, S_tok], F32R, tag="kt", name="kt")
                    for dst, w, w9 in ((qt, wq, wq9), (kt_t, wk, wk9)):
                        for half in range(S_tok // 1024):
                            psq = pssp.tile([128, 1024], F32, tag="pss",
                                            name="psq")
                            # k-outer: each w[k] weight load serves 2 matmuls
                            for k in range(KT):
                                for sub in range(2):
                                    ch = half * 2 + sub
                                    o = sub * 512
                                    nc.tensor.matmul(
                                        psq[:, o:o + 512], w[k],
                                        xtr[k][:, ch * 512:(ch + 1) * 512],
                                        start=(k == 0), stop=False)
                            for sub in range(2):
                                o = sub * 512
                                nc.tensor.matmul(
                                    psq[:, o:o + 512], w9, ones_row,
                                    start=False, stop=True)
                            nc.vector.tensor_copy(
                                dst[:, half * 1024:(half + 1) * 1024], psq)

                    # prefetch next head-pair's weights now, so their
                    # DMAs aren't queued behind this pair's evacuations
                    if hp + 1 < HPAIRS:
                        wnext = fetch_w(hp + 1)

                    # attention for the 2 heads of this pair.
                    # PV results are staged UNNORMALIZED (plus the Z row);
                    # the divide by Z runs on SBUF tiles only, off the PE
                    # critical path, so PSUM slots free immediately.
                    for icp in range(IC // 2):
                        ics = [2 * icp, 2 * icp + 1]
                        psys = {}
                        for ic in ics:
                            psys[(ic, 0)] = psyp.tile([65, 512], F32,
                                                      tag="psy", name="psyA")
                            psys[(ic, 1)] = psyp.tile([65, 512], F32,
                                                      tag="psy", name="psyB")
                        # j-tiles shared across the ic pair: each kT / v
                        # weight load serves both i-chunks
                        for jt in range(4 * ics[1] + 4):
                            ets = {}
                            for ic in ics:
                                if jt >= 4 * ic + 4:
                                    continue
                                pss = pssp.tile([128, 1024], F32, tag="pss",
                                                name="pss")
                                nc.tensor.matmul(
                                    pss[:, 0:512],
                                    kt_t[0:64, jt * 128:(jt + 1) * 128],
                                    qt[0:64, ic * 512:(ic + 1) * 512],
                                    start=True, stop=True,
                                    tile_position=(0, 0))
                                nc.tensor.matmul(
                                    pss[:, 512:1024],
                                    kt_t[64:128, jt * 128:(jt + 1) * 128],
                                    qt[64:128, ic * 512:(ic + 1) * 512],
                                    start=True, stop=True,
                                    tile_position=(64, 0))
                                et = hsb.tile([128, 1024], F32R, tag="et",
                                              bufs=3, name="et")
                                nc.scalar.activation(et, pss, ActF.Exp,
                                                     scale=0.125)
                                tdx = jt - 4 * ic
                                if tdx >= 0:
                                    w_ = 128 * (tdx + 1)
                                    nc.vector.tensor_mul(
                                        et[:, 0:w_], et[:, 0:w_], masks[tdx])
                                    nc.vector.tensor_mul(
                                        et[:, 512:512 + w_],
                                        et[:, 512:512 + w_], masks[tdx])
                                ets[ic] = et
                            for head in range(2):
                                vsl = v_sb[jt][:, (2 * hp + head) * 65:
                                               (2 * hp + head) * 65 + 65]
                                for ic in ics:
                                    if jt >= 4 * ic + 4:
                                        continue
                                    nc.tensor.matmul(
                                        psys[(ic, head)], vsl,
                                        ets[ic][:, head * 512:
                                                (head + 1) * 512],
                                        start=(jt == 0),
                                        stop=(jt == 4 * ic + 3))
                        for ic in ics:
                            # fast PSUM evacuation + SBUF-only normalize
                            sl = slice(ic * 512, (ic + 1) * 512)
                            zc = hsb.tile([1, 1024], F32, tag="zc", bufs=1,
                                          name="zc")
                            for head in range(2):
                                t65 = hsb.tile([65, 512], F32R, tag="t65",
                                               bufs=2, name="t65")
                                nc.scalar.activation(t65, psys[(ic, head)],
                                                     ActF.Copy)
                                nc.sync.dma_start(
                                    yT[hp][head * 64:(head + 1) * 64, sl],
                                    t65[0:64, :])
                                nc.sync.dma_start(
                                    zc[0:1, head * 512:(head + 1) * 512]
                                    .bitcast(F32R), t65[64:65, :])
                            nc.vector.reciprocal(zc, zc)
                            bcf = hsb.tile([128, 512], F32, tag="bcf",
                                           bufs=2, name="bcf")
                            nc.gpsimd.partition_broadcast(
                                bcf, zc[0:1, 512:1024])
                            nc.gpsimd.partition_broadcast(
                                bcf[0:64, :], zc[0:1, 0:512])
                            nc.vector.tensor_mul(yT[hp][:, sl],
                                                 yT[hp][:, sl], bcf)

                # ---- projection (inside the same pools: pso tiles come
                # from the pss pool so no PSUM pool boundary blocks it) ----
                with tc.tile_pool(name="wpp", bufs=1) as wpp, \
                     tc.tile_pool(name="osb", bufs=1) as osb:
                    wp = []
                    for k in range(HPAIRS):
                        t = wpp.tile([128, D], F32R, name=f"wp{k}")
                        nc.sync.dma_start(
                            t, Wp[k * 128:(k + 1) * 128, :].bitcast(F32R))
                        wp.append(t)
                    for tt in range(NT):
                        pso = pssp.tile([128, 1024], F32, tag="pss",
                                        name="pso")
                        # k-outer: each yT weight load serves both col chunks
                        for k in range(HPAIRS):
                            for nch in range(2):
                                nc.tensor.matmul(
                                    pso[:, nch * 512:(nch + 1) * 512],
                                    yT[k][:, tt * 128:(tt + 1) * 128],
                                    wp[k][:, nch * 512:(nch + 1) * 512],
                                    start=(k == 0), stop=(k == HPAIRS - 1))
                        for nch in range(2):
                            ot = osb.tile([128, 512], F32, tag="ot",
                                          bufs=3, name="ot")
                            nc.scalar.activation(
                                ot, pso[:, nch * 512:(nch + 1) * 512],
                                ActF.Copy)
                            nc.sync.dma_start(
                                out[tt * 128:(tt + 1) * 128,
                                    nch * 512:(nch + 1) * 512], ot)
    nc.finalize()
    _nc_cache[key] = nc
    return nc


def make_in_maps(x, W_attn, b_attn, W_proj):
    """Build per-core input dicts from full inputs."""
    Bx, Sx, Dx = x.shape
    in_maps = []
    for c in range(N_CORES):
        b = c // 2
        g = c % 2
        cs = slice(g * 512, (g + 1) * 512)
        xT_aug = np.ascontiguousarray(x[b].T)
        wq = np.concatenate([W_attn[:, 0:D][:, cs],
                             b_attn[0:D][cs][None, :]], axis=0)
        wk = np.concatenate([W_attn[:, D:2 * D][:, cs],
                             b_attn[D:2 * D][cs][None, :]], axis=0)
        wv = np.concatenate([W_attn[:, 2 * D:3 * D][:, cs],
                             b_attn[2 * D:3 * D][cs][None, :]], axis=0)
        wp = np.ascontiguousarray(W_proj[cs, :])
        in_maps.append({
            "xT": np.ascontiguousarray(xT_aug, dtype=np.float32),
            "Wq": np.ascontiguousarray(wq, dtype=np.float32),
            "Wk": np.ascontiguousarray(wk, dtype=np.float32),
            "Wv": np.ascontiguousarray(wv, dtype=np.float32),
            "Wp": wp.astype(np.float32),
        })
    return in_maps


def kernel(x, W_attn, b_attn, W_proj, b_proj, trace=False):
    x = np.asarray(x, dtype=np.float32)
    W_attn = np.asarray(W_attn, dtype=np.float32)
    b_attn = np.asarray(b_attn, dtype=np.float32)
    W_proj = np.asarray(W_proj, dtype=np.float32)
    b_proj = np.asarray(b_proj, dtype=np.float32)
    nc = build_nc(x.shape[1], N_CORES)
    in_maps = make_in_maps(x, W_attn, b_attn, W_proj)
    res = bass_utils.run_bass_kernel_spmd(
        nc, in_maps, core_ids=list(range(N_CORES)), trace=trace)
    Bx, Sx, Dx = x.shape
    outp = np.empty((Bx, Sx, Dx), dtype=np.float32)
    for b in range(Bx):
        outp[b] = (res.results[2 * b]["out"] + res.results[2 * b + 1]["out"]
                   + b_proj[None, :])
    if trace:
        return outp, res
    return outp


# revision 26
# speedup vs baseline: 1.2220x; 1.2220x over previous
"""Causal self-attention kernel for 8 trn2 NeuronCores.

Sharding: 4 batches x 2 head-groups (8 heads each). Core c handles
batch c//2, heads (c%2)*8 .. (c%2)*8+8. Each core computes qkv for its
head-group, causal attention, and a partial projection; the host sums
the two head-group partials per batch and adds b_proj.

Device math (per core, all matmuls fp32r):
  qT[c,t]  = sum_k Wq_aug[k,c] xT_aug[k,t]   (bias folded via ones row)
  kT, v    similarly (v in natural [token, col] layout, with an
            interleaved ones column per head for the softmax denominator)
  scoresT[j,i] = kT.T q / 8   per head, exp'd on ScalarE (no max
            subtraction needed: scores ~ N(0,1)), causal-masked by
            multiplying precomputed 0/1 masks on the diagonal blocks
  yT_un[d,i], Z[i] = PV matmul with ones column (m=65)
  yT = yT_un * (1/Z) broadcast (gpsimd partition_broadcast), off the
            PE critical path so PSUM slots recycle fast (keeps HAM warm)
  outT partial = yT.T @ Wp   (natural [token, col] output)
"""

import sys
import os

for _p in ("/opt/trn_rl_repo", "/root/.axon_site/_ro/trn_rl_repo"):
    if os.path.isdir(_p) and _p not in sys.path:
        sys.path.insert(0, _p)

import numpy as np
import concourse.bass as bass  # noqa: F401
import concourse.mybir as mybir
import concourse.tile as tile
from concourse import bacc, bass_utils

F32 = mybir.dt.float32
F32R = mybir.dt.float32r
ActF = mybir.ActivationFunctionType

B, S, D, H = 4, 2048, 1024, 16
NH = 8          # heads per core
HPAIRS = NH // 2
KT = D // 128   # 8 k-tiles over D
N_CORES = 8

_nc_cache = {}


def build_nc(S_tok=S, n_cores=N_CORES):
    key = (S_tok, n_cores)
    if key in _nc_cache:
        return _nc_cache[key]
    IC = S_tok // 512      # query chunks
    NT = S_tok // 128      # token tiles
    nc = bacc.Bacc("TRN2", target_bir_lowering=False, debug=False,
                   num_devices=n_cores)
    xT = nc.dram_tensor("xT", [D, S_tok], F32, kind="ExternalInput").ap()
    Wq = nc.dram_tensor("Wq", [D + 1, 512], F32, kind="ExternalInput").ap()
    Wk = nc.dram_tensor("Wk", [D + 1, 512], F32, kind="ExternalInput").ap()
    Wv = nc.dram_tensor("Wv", [D + 1, 512], F32, kind="ExternalInput").ap()
    Wp = nc.dram_tensor("Wp", [512, D], F32, kind="ExternalInput").ap()
    out = nc.dram_tensor("out", [S_tok, D], F32, kind="ExternalOutput").ap()

    with tile.TileContext(nc) as tc:
        with tc.tile_pool(name="persist", bufs=1) as pp:
            # resident xT (f32r views of the f32 dram rows); DMAs are
            # issued inside the V-pass scope, interleaved with the Wv
            # loads and column-chunked so the first matmuls start early
            xtr = [pp.tile([128, S_tok], F32R, name=f"xtr{k}")
                   for k in range(KT)]
            # constant ones row: rhs/lhsT for the folded-bias K=1 matmuls
            ones_f32 = pp.tile([1, 512], F32, name="ones_f32")
            nc.gpsimd.memset(ones_f32, 1.0)
            ones_row = pp.tile([1, 512], F32R, name="ones_row")
            nc.vector.tensor_copy(ones_row, ones_f32)

            # v in natural layout, 65-stride per head (64 v cols + ones col)
            v_sb = [pp.tile([128, 8 * 65], F32R, name=f"vsb{t}")
                    for t in range(NT)]
            # yT accumulation per head pair [local d, tokens]
            yT = [pp.tile([128, S_tok], F32R, name=f"ytr{h}")
                  for h in range(HPAIRS)]
            # causal masks for the 4 diagonal offsets r = 128*t:
            # mask[p, y] = 1 if y >= p + r else 0
            masks = []
            BF16 = mybir.dt.bfloat16
            for t in range(4):
                wm = 128 * (t + 1)
                m = pp.tile([128, wm], BF16, name=f"mask{t}")
                nc.gpsimd.memset(m, 1.0)
                nc.gpsimd.affine_select(
                    out=m, in_=m, compare_op=mybir.AluOpType.is_ge,
                    fill=0.0, base=-128 * t, pattern=[[1, wm]],
                    channel_multiplier=-1)
                masks.append(m)
            ones8 = pp.tile([128, 8, 1], F32, name="ones8")
            nc.gpsimd.memset(ones8, 1.0)

            # ---- V pass ----
            with tc.tile_pool(name="wv", bufs=1) as wvp, \
                 tc.tile_pool(name="ps1", bufs=4, space="PSUM") as ps1:
                wv = [wvp.tile([128, 512], F32R, name=f"wv{k}")
                      for k in range(KT)]
                wv9 = wvp.tile([1, 512], F32R, name="wv9")
                nc.sync.dma_start(wv9, Wv[D:D + 1, :].bitcast(F32R))
                for k in range(KT):
                    nc.sync.dma_start(
                        wv[k], Wv[k * 128:(k + 1) * 128, :].bitcast(F32R))
                    nc.sync.dma_start(
                        xtr[k][:, 0:512],
                        xT[k * 128:(k + 1) * 128, 0:512].bitcast(F32R))
                for c in range(1, S_tok // 512):
                    cs = slice(c * 512, (c + 1) * 512)
                    for k in range(KT):
                        nc.sync.dma_start(
                            xtr[k][:, cs],
                            xT[k * 128:(k + 1) * 128, cs].bitcast(F32R))
                for g in range(NT // 4):
                    psvs = [ps1.tile([128, 512], F32, tag="psv", name="psv")
                            for _ in range(4)]
                    # k-outer so each wv[k] weight load serves 4 matmuls
                    for k in range(KT):
                        for j in range(4):
                            t = g * 4 + j
                            nc.tensor.matmul(
                                psvs[j], xtr[k][:, t * 128:(t + 1) * 128],
                                wv[k], start=(k == 0), stop=False)
                    for j in range(4):
                        nc.tensor.matmul(psvs[j], ones_row[0:1, 0:128],
                                         wv9, start=False, stop=True)
                    for j in range(4):
                        t = g * 4 + j
                        vv = v_sb[t].rearrange("p (h c) -> p h c", c=65)
                        nc.scalar.activation(
                            vv[:, :, 0:64],
                            psvs[j].rearrange("p (h c) -> p h c", c=64),
                            ActF.Copy)
                        nc.vector.tensor_copy(vv[:, :, 64:65], ones8)

            # ---- per head-pair: q/k then attention ----
            with tc.tile_pool(name="hsb", bufs=1) as hsb, \
                 tc.tile_pool(name="pss", bufs=2, space="PSUM") as pssp, \
                 tc.tile_pool(name="psy", bufs=4, space="PSUM") as psyp:
                def fetch_w(hp):
                    wq, wk = [], []
                    for k in range(KT):
                        tq = hsb.tile([128, 128], F32R, tag=f"wq{k}",
                                      name="wq")
                        nc.sync.dma_start(
                            tq, Wq[k * 128:(k + 1) * 128,
                                   hp * 128:(hp + 1) * 128].bitcast(F32R))
                        wq.append(tq)
                        tk = hsb.tile([128, 128], F32R, tag=f"wk{k}",
                                      name="wk")
                        nc.sync.dma_start(
                            tk, Wk[k * 128:(k + 1) * 128,
                                   hp * 128:(hp + 1) * 128].bitcast(F32R))
                        wk.append(tk)
                    wq9 = hsb.tile([1, 128], F32R, tag="wq9", name="wq9")
                    nc.sync.dma_start(
                        wq9, Wq[D:D + 1,
                                hp * 128:(hp + 1) * 128].bitcast(F32R))
                    wk9 = hsb.tile([1, 128], F32R, tag="wk9", name="wk9")
                    nc.sync.dma_start(
                        wk9, Wk[D:D + 1,
                                hp * 128:(hp + 1) * 128].bitcast(F32R))
                    return wq, wk, wq9, wk9

                wnext = fetch_w(0)
                for hp in range(HPAIRS):
                    wq, wk, wq9, wk9 = wnext

                    qt = hsb.tile([128, S_tok], F32R, tag="qt", name="qt")
                    kt_t = hsb.tile([128, S_tok], F32R, tag="kt", name="kt")
                    for dst, w, w9 in ((qt, wq, wq9), (kt_t, wk, wk9)):
                        for half in range(S_tok // 1024):
                            psq = pssp.tile([128, 1024], F32, tag="pss",
                                            name="psq")
                            # k-outer: each w[k] weight load serves 2 matmuls
                            for k in range(KT):
                                for sub in range(2):
                                    ch = half * 2 + sub
                                    o = sub * 512
                                    nc.tensor.matmul(
                                        psq[:, o:o + 512], w[k],
                                        xtr[k][:, ch * 512:(ch + 1) * 512],
                                        start=(k == 0), stop=False)
                            for sub in range(2):
                                o = sub * 512
                                nc.tensor.matmul(
                                    psq[:, o:o + 512], w9, ones_row,
                                    start=False, stop=True)
                            nc.scalar.activation(
                                dst[:, half * 1024:(half + 1) * 1024], psq,
                                ActF.Copy)

                    # prefetch next head-pair's weights now, so their
                    # DMAs aren't queued behind this pair's evacuations
                    if hp + 1 < HPAIRS:
                        wnext = fetch_w(hp + 1)

                    # attention for the 2 heads of this pair.
                    # PV results are staged UNNORMALIZED (plus the Z row);
                    # the divide by Z runs on SBUF tiles only, off the PE
                    # critical path, so PSUM slots free immediately.
                    for icp in range(IC // 2):
                        ics = [2 * icp, 2 * icp + 1]
                        psys = {}
                        for ic in ics:
                            psys[(ic, 0)] = psyp.tile([65, 512], F32,
                                                      tag="psy", name="psyA")
                            psys[(ic, 1)] = psyp.tile([65, 512], F32,
                                                      tag="psy", name="psyB")
                        # j-tiles shared across the ic pair: each kT / v
                        # weight load serves both i-chunks
                        for jt in range(4 * ics[1] + 4):
                            ets = {}
                            for ic in ics:
                                if jt >= 4 * ic + 4:
                                    continue
                                pss = pssp.tile([128, 1024], F32, tag="pss",
                                                name="pss")
                                nc.tensor.matmul(
                                    pss[:, 0:512],
                                    kt_t[0:64, jt * 128:(jt + 1) * 128],
                                    qt[0:64, ic * 512:(ic + 1) * 512],
                                    start=True, stop=True,
                                    tile_position=(0, 0))
                                nc.tensor.matmul(
                                    pss[:, 512:1024],
                                    kt_t[64:128, jt * 128:(jt + 1) * 128],
                                    qt[64:128, ic * 512:(ic + 1) * 512],
                                    start=True, stop=True,
                                    tile_position=(64, 0))
                                et = hsb.tile([128, 1024], F32R, tag="et",
                                              bufs=3, name="et")
                                nc.scalar.activation(et, pss, ActF.Exp,
                                                     scale=0.125)
                                tdx = jt - 4 * ic
                                if tdx >= 0:
                                    w_ = 128 * (tdx + 1)
                                    nc.vector.tensor_mul(
                                        et[:, 0:w_], et[:, 0:w_], masks[tdx])
                                    nc.vector.tensor_mul(
                                        et[:, 512:512 + w_],
                                        et[:, 512:512 + w_], masks[tdx])
                                ets[ic] = et
                            for head in range(2):
                                vsl = v_sb[jt][:, (2 * hp + head) * 65:
                                               (2 * hp + head) * 65 + 65]
                                for ic in ics:
                                    if jt >= 4 * ic + 4:
                                        continue
                                    nc.tensor.matmul(
                                        psys[(ic, head)], vsl,
                                        ets[ic][:, head * 512:
                                                (head + 1) * 512],
                                        start=(jt == 0),
                                        stop=(jt == 4 * ic + 3))
                        for ic in ics:
                            # fast PSUM evacuation + SBUF-only normalize
                            sl = slice(ic * 512, (ic + 1) * 512)
                            zc = hsb.tile([1, 1024], F32, tag="zc", bufs=1,
                                          name="zc")
                            for head in range(2):
                                t65 = hsb.tile([65, 512], F32R, tag="t65",
                                               bufs=2, name="t65")
                                nc.scalar.activation(t65, psys[(ic, head)],
                                                     ActF.Copy)
                                nc.sync.dma_start(
                                    yT[hp][head * 64:(head + 1) * 64, sl],
                                    t65[0:64, :])
                                nc.sync.dma_start(
                                    zc[0:1, head * 512:(head + 1) * 512]
                                    .bitcast(F32R), t65[64:65, :])
                            nc.vector.reciprocal(zc, zc)
                            bcf = hsb.tile([128, 512], F32, tag="bcf",
                                           bufs=2, name="bcf")
                            nc.gpsimd.partition_broadcast(
                                bcf, zc[0:1, 512:1024])
                            nc.gpsimd.partition_broadcast(
                                bcf[0:64, :], zc[0:1, 0:512])
                            nc.vector.tensor_mul(yT[hp][:, sl],
                                                 yT[hp][:, sl], bcf)

                # ---- projection (inside the same pools: pso tiles come
                # from the pss pool so no PSUM pool boundary blocks it) ----
                with tc.tile_pool(name="wpp", bufs=1) as wpp, \
                     tc.tile_pool(name="osb", bufs=1) as osb:
                    wp = []
                    for k in range(HPAIRS):
                        t = wpp.tile([128, D], F32R, name=f"wp{k}")
                        nc.sync.dma_start(
                            t, Wp[k * 128:(k + 1) * 128, :].bitcast(F32R))
                        wp.append(t)
                    for tt in range(NT):
                        pso = pssp.tile([128, 1024], F32, tag="pss",
                                        name="pso")
                        # k-outer: each yT weight load serves both col chunks
                        for k in range(HPAIRS):
                            for nch in range(2):
                                nc.tensor.matmul(
                                    pso[:, nch * 512:(nch + 1) * 512],
                                    yT[k][:, tt * 128:(tt + 1) * 128],
                                    wp[k][:, nch * 512:(nch + 1) * 512],
                                    start=(k == 0), stop=(k == HPAIRS - 1))
                        for nch in range(2):
                            ot = osb.tile([128, 512], F32, tag="ot",
                                          bufs=3, name="ot")
                            nc.scalar.activation(
                                ot, pso[:, nch * 512:(nch + 1) * 512],
                                ActF.Copy)
                            nc.sync.dma_start(
                                out[tt * 128:(tt + 1) * 128,
                                    nch * 512:(nch + 1) * 512], ot)
    nc.finalize()
    _nc_cache[key] = nc
    return nc


def make_in_maps(x, W_attn, b_attn, W_proj):
    """Build per-core input dicts from full inputs."""
    Bx, Sx, Dx = x.shape
    in_maps = []
    for c in range(N_CORES):
        b = c // 2
        g = c % 2
        cs = slice(g * 512, (g + 1) * 512)
        xT_aug = np.ascontiguousarray(x[b].T)
        wq = np.concatenate([W_attn[:, 0:D][:, cs],
                             b_attn[0:D][cs][None, :]], axis=0)
        wk = np.concatenate([W_attn[:, D:2 * D][:, cs],
                             b_attn[D:2 * D][cs][None, :]], axis=0)
        wv = np.concatenate([W_attn[:, 2 * D:3 * D][:, cs],
                             b_attn[2 * D:3 * D][cs][None, :]], axis=0)
        wp = np.ascontiguousarray(W_proj[cs, :])
        in_maps.append({
            "xT": np.ascontiguousarray(xT_aug, dtype=np.float32),
            "Wq": np.ascontiguousarray(wq, dtype=np.float32),
            "Wk": np.ascontiguousarray(wk, dtype=np.float32),
            "Wv": np.ascontiguousarray(wv, dtype=np.float32),
            "Wp": wp.astype(np.float32),
        })
    return in_maps


def kernel(x, W_attn, b_attn, W_proj, b_proj, trace=False):
    x = np.asarray(x, dtype=np.float32)
    W_attn = np.asarray(W_attn, dtype=np.float32)
    b_attn = np.asarray(b_attn, dtype=np.float32)
    W_proj = np.asarray(W_proj, dtype=np.float32)
    b_proj = np.asarray(b_proj, dtype=np.float32)
    nc = build_nc(x.shape[1], N_CORES)
    in_maps = make_in_maps(x, W_attn, b_attn, W_proj)
    res = bass_utils.run_bass_kernel_spmd(
        nc, in_maps, core_ids=list(range(N_CORES)), trace=trace)
    Bx, Sx, Dx = x.shape
    outp = np.empty((Bx, Sx, Dx), dtype=np.float32)
    for b in range(Bx):
        outp[b] = (res.results[2 * b]["out"] + res.results[2 * b + 1]["out"]
                   + b_proj[None, :])
    if trace:
        return outp, res
    return outp


# revision 27
# speedup vs baseline: 1.3108x; 1.0727x over previous
"""Causal self-attention kernel for 8 trn2 NeuronCores.

Sharding: 4 batches x 2 head-groups (8 heads each). Core c handles
batch c//2, heads (c%2)*8 .. (c%2)*8+8. Each core computes qkv for its
head-group, causal attention, and a partial projection; the host sums
the two head-group partials per batch and adds b_proj.

Device math (per core, all matmuls fp32r):
  qT[c,t]  = sum_k Wq_aug[k,c] xT_aug[k,t]   (bias folded via ones row)
  kT, v    similarly (v in natural [token, col] layout, with an
            interleaved ones column per head for the softmax denominator)
  scoresT[j,i] = kT.T q / 8   per head, exp'd on ScalarE (no max
            subtraction needed: scores ~ N(0,1)), causal-masked by
            multiplying precomputed 0/1 masks on the diagonal blocks
  yT_un[d,i], Z[i] = PV matmul with ones column (m=65)
  yT = yT_un * (1/Z) broadcast (gpsimd partition_broadcast), off the
            PE critical path so PSUM slots recycle fast (keeps HAM warm)
  outT partial = yT.T @ Wp   (natural [token, col] output)
"""

import sys
import os

for _p in ("/opt/trn_rl_repo", "/root/.axon_site/_ro/trn_rl_repo"):
    if os.path.isdir(_p) and _p not in sys.path:
        sys.path.insert(0, _p)

import numpy as np
import concourse.bass as bass  # noqa: F401
import concourse.mybir as mybir
import concourse.tile as tile
from concourse import bacc, bass_utils

F32 = mybir.dt.float32
F32R = mybir.dt.float32r
ActF = mybir.ActivationFunctionType

B, S, D, H = 4, 2048, 1024, 16
NH = 8          # heads per core
HPAIRS = NH // 2
KT = D // 128   # 8 k-tiles over D
N_CORES = 8

_nc_cache = {}


def build_nc(S_tok=S, n_cores=N_CORES):
    key = (S_tok, n_cores)
    if key in _nc_cache:
        return _nc_cache[key]
    IC = S_tok // 512      # query chunks
    NT = S_tok // 128      # token tiles
    nc = bacc.Bacc("TRN2", target_bir_lowering=False, debug=False,
                   num_devices=n_cores)
    xT = nc.dram_tensor("xT", [D, S_tok], F32, kind="ExternalInput").ap()
    Wq = nc.dram_tensor("Wq", [D + 1, 512], F32, kind="ExternalInput").ap()
    Wk = nc.dram_tensor("Wk", [D + 1, 512], F32, kind="ExternalInput").ap()
    Wv = nc.dram_tensor("Wv", [D + 1, 512], F32, kind="ExternalInput").ap()
    Wp = nc.dram_tensor("Wp", [512, D], F32, kind="ExternalInput").ap()
    out = nc.dram_tensor("out", [S_tok, D], F32, kind="ExternalOutput").ap()

    with tile.TileContext(nc) as tc:
        with tc.tile_pool(name="persist", bufs=1) as pp:
            # resident xT (f32r views of the f32 dram rows); DMAs are
            # issued inside the V-pass scope, interleaved with the Wv
            # loads and column-chunked so the first matmuls start early
            xtr = [pp.tile([128, S_tok], F32R, name=f"xtr{k}")
                   for k in range(KT)]
            # constant ones row: rhs/lhsT for the folded-bias K=1 matmuls
            ones_f32 = pp.tile([1, 512], F32, name="ones_f32")
            nc.gpsimd.memset(ones_f32, 1.0)
            ones_row = pp.tile([1, 512], F32R, name="ones_row")
            nc.vector.tensor_copy(ones_row, ones_f32)

            # v in natural layout, 65-stride per head (64 v cols + ones col)
            v_sb = [pp.tile([128, 8 * 65], F32R, name=f"vsb{t}")
                    for t in range(NT)]
            # yT accumulation per head pair [local d, tokens]
            yT = [pp.tile([128, S_tok], F32R, name=f"ytr{h}")
                  for h in range(HPAIRS)]
            # causal masks for the 4 diagonal offsets r = 128*t:
            # mask[p, y] = 1 if y >= p + r else 0
            masks = []
            BF16 = mybir.dt.bfloat16
            for t in range(4):
                wm = 128 * (t + 1)
                m = pp.tile([128, wm], BF16, name=f"mask{t}")
                nc.gpsimd.memset(m, 1.0)
                nc.gpsimd.affine_select(
                    out=m, in_=m, compare_op=mybir.AluOpType.is_ge,
                    fill=0.0, base=-128 * t, pattern=[[1, wm]],
                    channel_multiplier=-1)
                masks.append(m)
            ones8 = pp.tile([128, 8, 1], F32, name="ones8")
            nc.gpsimd.memset(ones8, 1.0)

            # ---- V pass ----
            with tc.tile_pool(name="wv", bufs=1) as wvp, \
                 tc.tile_pool(name="ps1", bufs=4, space="PSUM") as ps1:
                wv = [wvp.tile([128, 512], F32R, name=f"wv{k}")
                      for k in range(KT)]
                wv9 = wvp.tile([1, 512], F32R, name="wv9")
                nc.sync.dma_start(wv9, Wv[D:D + 1, :].bitcast(F32R))
                for k in range(KT):
                    nc.sync.dma_start(
                        wv[k], Wv[k * 128:(k + 1) * 128, :].bitcast(F32R))
                    nc.sync.dma_start(
                        xtr[k][:, 0:512],
                        xT[k * 128:(k + 1) * 128, 0:512].bitcast(F32R))
                for c in range(1, S_tok // 512):
                    cs = slice(c * 512, (c + 1) * 512)
                    for k in range(KT):
                        nc.sync.dma_start(
                            xtr[k][:, cs],
                            xT[k * 128:(k + 1) * 128, cs].bitcast(F32R))
                for g in range(NT // 4):
                    psvs = [ps1.tile([128, 512], F32, tag="psv", name="psv")
                            for _ in range(4)]
                    # k-outer so each wv[k] weight load serves 4 matmuls
                    for k in range(KT):
                        for j in range(4):
                            t = g * 4 + j
                            nc.tensor.matmul(
                                psvs[j], xtr[k][:, t * 128:(t + 1) * 128],
                                wv[k], start=(k == 0), stop=False)
                    for j in range(4):
                        nc.tensor.matmul(psvs[j], ones_row[0:1, 0:128],
                                         wv9, start=False, stop=True)
                    for j in range(4):
                        t = g * 4 + j
                        vv = v_sb[t].rearrange("p (h c) -> p h c", c=65)
                        nc.scalar.activation(
                            vv[:, :, 0:64],
                            psvs[j].rearrange("p (h c) -> p h c", c=64),
                            ActF.Copy)
                        nc.vector.tensor_copy(vv[:, :, 64:65], ones8)

            # ---- per head-pair: q/k then attention ----
            with tc.tile_pool(name="hsb", bufs=1) as hsb, \
                 tc.tile_pool(name="pss", bufs=2, space="PSUM") as pssp, \
                 tc.tile_pool(name="psy", bufs=4, space="PSUM") as psyp:
                def fetch_w(hp):
                    wq, wk = [], []
                    for k in range(KT):
                        tq = hsb.tile([128, 128], F32R, tag=f"wq{k}",
                                      name="wq")
                        nc.sync.dma_start(
                            tq, Wq[k * 128:(k + 1) * 128,
                                   hp * 128:(hp + 1) * 128].bitcast(F32R))
                        wq.append(tq)
                        tk = hsb.tile([128, 128], F32R, tag=f"wk{k}",
                                      name="wk")
                        nc.sync.dma_start(
                            tk, Wk[k * 128:(k + 1) * 128,
                                   hp * 128:(hp + 1) * 128].bitcast(F32R))
                        wk.append(tk)
                    wq9 = hsb.tile([1, 128], F32R, tag="wq9", name="wq9")
                    nc.sync.dma_start(
                        wq9, Wq[D:D + 1,
                                hp * 128:(hp + 1) * 128].bitcast(F32R))
                    wk9 = hsb.tile([1, 128], F32R, tag="wk9", name="wk9")
                    nc.sync.dma_start(
                        wk9, Wk[D:D + 1,
                                hp * 128:(hp + 1) * 128].bitcast(F32R))
                    return wq, wk, wq9, wk9

                wnext = fetch_w(0)
                for hp in range(HPAIRS):
                    wq, wk, wq9, wk9 = wnext

                    qt = hsb.tile([128, S_tok], F32R, tag="qt", name="qt")
                    kt_t = hsb.tile([128, S_tok], F32R, tag="kt", name="kt")
                    for dst, w, w9 in ((qt, wq, wq9), (kt_t, wk, wk9)):
                        for half in range(S_tok // 1024):
                            psq = pssp.tile([128, 1024], F32, tag="pss",
                                            name="psq")
                            # k-outer: each w[k] weight load serves 2 matmuls
                            for k in range(KT):
                                for sub in range(2):
                                    ch = half * 2 + sub
                                    o = sub * 512
                                    nc.tensor.matmul(
                                        psq[:, o:o + 512], w[k],
                                        xtr[k][:, ch * 512:(ch + 1) * 512],
                                        start=(k == 0), stop=False)
                            for sub in range(2):
                                o = sub * 512
                                nc.tensor.matmul(
                                    psq[:, o:o + 512], w9, ones_row,
                                    start=False, stop=True)
                            nc.scalar.activation(
                                dst[:, half * 1024:(half + 1) * 1024], psq,
                                ActF.Copy)

                    # prefetch next head-pair's weights now, so their
                    # DMAs aren't queued behind this pair's evacuations
                    if hp + 1 < HPAIRS:
                        wnext = fetch_w(hp + 1)

                    # attention for the 2 heads of this pair.
                    # PV results are staged UNNORMALIZED (plus the Z row);
                    # the divide by Z runs on SBUF tiles only, off the PE
                    # critical path, so PSUM slots free immediately.
                    for icp in range(IC // 2):
                        ics = [2 * icp, 2 * icp + 1]
                        psys = {}
                        for ic in ics:
                            psys[(ic, 0)] = psyp.tile([65, 512], F32,
                                                      tag="psy", name="psyA")
                            psys[(ic, 1)] = psyp.tile([65, 512], F32,
                                                      tag="psy", name="psyB")
                        # j-tiles shared across the ic pair: each kT / v
                        # weight load serves both i-chunks
                        for jt in range(4 * ics[1] + 4):
                            ets = {}
                            for ic in ics:
                                if jt >= 4 * ic + 4:
                                    continue
                                pss = pssp.tile([128, 1024], F32, tag="pss",
                                                name="pss")
                                nc.tensor.matmul(
                                    pss[:, 0:512],
                                    kt_t[0:64, jt * 128:(jt + 1) * 128],
                                    qt[0:64, ic * 512:(ic + 1) * 512],
                                    start=True, stop=True,
                                    tile_position=(0, 0))
                                nc.tensor.matmul(
                                    pss[:, 512:1024],
                                    kt_t[64:128, jt * 128:(jt + 1) * 128],
                                    qt[64:128, ic * 512:(ic + 1) * 512],
                                    start=True, stop=True,
                                    tile_position=(64, 0))
                                et = hsb.tile([128, 1024], F32R, tag="et",
                                              bufs=3, name="et")
                                nc.scalar.activation(et, pss, ActF.Exp,
                                                     scale=0.125)
                                tdx = jt - 4 * ic
                                if tdx >= 0:
                                    w_ = 128 * (tdx + 1)
                                    nc.vector.tensor_mul(
                                        et[:, 0:w_], et[:, 0:w_], masks[tdx])
                                    nc.vector.tensor_mul(
                                        et[:, 512:512 + w_],
                                        et[:, 512:512 + w_], masks[tdx])
                                ets[ic] = et
                            for head in range(2):
                                vsl = v_sb[jt][:, (2 * hp + head) * 65:
                                               (2 * hp + head) * 65 + 65]
                                for ic in ics:
                                    if jt >= 4 * ic + 4:
                                        continue
                                    nc.tensor.matmul(
                                        psys[(ic, head)], vsl,
                                        ets[ic][:, head * 512:
                                                (head + 1) * 512],
                                        start=(jt == 0),
                                        stop=(jt == 4 * ic + 3))
                        for ic in ics:
                            # fast PSUM evacuation + SBUF-only normalize
                            sl = slice(ic * 512, (ic + 1) * 512)
                            zc = hsb.tile([1, 1024], F32, tag="zc", bufs=1,
                                          name="zc")
                            for head in range(2):
                                t65 = hsb.tile([65, 512], F32R, tag="t65",
                                               bufs=2, name="t65")
                                nc.scalar.activation(t65, psys[(ic, head)],
                                                     ActF.Copy)
                                nc.sync.dma_start(
                                    yT[hp][head * 64:(head + 1) * 64, sl],
                                    t65[0:64, :])
                                nc.sync.dma_start(
                                    zc[0:1, head * 512:(head + 1) * 512]
                                    .bitcast(F32R), t65[64:65, :])
                            # reciprocal at full lane width: scatter the
                            # 1024 Z values over 128 partitions, recip,
                            # gather back (1-lane recip costs 6.5us)
                            zs = hsb.tile([128, 8], F32, tag="zs", bufs=2,
                                          name="zs")
                            nc.sync.dma_start(zs, zc)
                            nc.vector.reciprocal(zs, zs)
                            nc.sync.dma_start(zc, zs)
                            bcf = hsb.tile([128, 512], F32, tag="bcf",
                                           bufs=2, name="bcf")
                            nc.gpsimd.partition_broadcast(
                                bcf, zc[0:1, 512:1024])
                            nc.gpsimd.partition_broadcast(
                                bcf[0:64, :], zc[0:1, 0:512])
                            nc.vector.tensor_mul(yT[hp][:, sl],
                                                 yT[hp][:, sl], bcf)

                # ---- projection (inside the same pools: pso tiles come
                # from the pss pool so no PSUM pool boundary blocks it) ----
                with tc.tile_pool(name="wpp", bufs=1) as wpp, \
                     tc.tile_pool(name="osb", bufs=1) as osb:
                    wp = []
                    for k in range(HPAIRS):
                        t = wpp.tile([128, D], F32R, name=f"wp{k}")
                        nc.sync.dma_start(
                            t, Wp[k * 128:(k + 1) * 128, :].bitcast(F32R))
                        wp.append(t)
                    for tt in range(NT):
                        pso = pssp.tile([128, 1024], F32, tag="pss",
                                        name="pso")
                        # k-outer: each yT weight load serves both col chunks
                        for k in range(HPAIRS):
                            for nch in range(2):
                                nc.tensor.matmul(
                                    pso[:, nch * 512:(nch + 1) * 512],
                                    yT[k][:, tt * 128:(tt + 1) * 128],
                                    wp[k][:, nch * 512:(nch + 1) * 512],
                                    start=(k == 0), stop=(k == HPAIRS - 1))
                        for nch in range(2):
                            ot = osb.tile([128, 512], F32, tag="ot",
                                          bufs=3, name="ot")
                            nc.scalar.activation(
                                ot, pso[:, nch * 512:(nch + 1) * 512],
                                ActF.Copy)
                            nc.sync.dma_start(
                                out[tt * 128:(tt + 1) * 128,
                                    nch * 512:(nch + 1) * 512], ot)
    nc.finalize()
    _nc_cache[key] = nc
    return nc


def make_in_maps(x, W_attn, b_attn, W_proj):
    """Build per-core input dicts from full inputs."""
    Bx, Sx, Dx = x.shape
    in_maps = []
    for c in range(N_CORES):
        b = c // 2
        g = c % 2
        cs = slice(g * 512, (g + 1) * 512)
        xT_aug = np.ascontiguousarray(x[b].T)
        wq = np.concatenate([W_attn[:, 0:D][:, cs],
                             b_attn[0:D][cs][None, :]], axis=0)
        wk = np.concatenate([W_attn[:, D:2 * D][:, cs],
                             b_attn[D:2 * D][cs][None, :]], axis=0)
        wv = np.concatenate([W_attn[:, 2 * D:3 * D][:, cs],
                             b_attn[2 * D:3 * D][cs][None, :]], axis=0)
        wp = np.ascontiguousarray(W_proj[cs, :])
        in_maps.append({
            "xT": np.ascontiguousarray(xT_aug, dtype=np.float32),
            "Wq": np.ascontiguousarray(wq, dtype=np.float32),
            "Wk": np.ascontiguousarray(wk, dtype=np.float32),
            "Wv": np.ascontiguousarray(wv, dtype=np.float32),
            "Wp": wp.astype(np.float32),
        })
    return in_maps


def kernel(x, W_attn, b_attn, W_proj, b_proj, trace=False):
    x = np.asarray(x, dtype=np.float32)
    W_attn = np.asarray(W_attn, dtype=np.float32)
    b_attn = np.asarray(b_attn, dtype=np.float32)
    W_proj = np.asarray(W_proj, dtype=np.float32)
    b_proj = np.asarray(b_proj, dtype=np.float32)
    nc = build_nc(x.shape[1], N_CORES)
    in_maps = make_in_maps(x, W_attn, b_attn, W_proj)
    res = bass_utils.run_bass_kernel_spmd(
        nc, in_maps, core_ids=list(range(N_CORES)), trace=trace)
    Bx, Sx, Dx = x.shape
    outp = np.empty((Bx, Sx, Dx), dtype=np.float32)
    for b in range(Bx):
        outp[b] = (res.results[2 * b]["out"] + res.results[2 * b + 1]["out"]
                   + b_proj[None, :])
    if trace:
        return outp, res
    return outp


# revision 28
# speedup vs baseline: 1.3343x; 1.0179x over previous
"""Causal self-attention kernel for 8 trn2 NeuronCores.

Sharding: 4 batches x 2 head-groups (8 heads each). Core c handles
batch c//2, heads (c%2)*8 .. (c%2)*8+8. Each core computes qkv for its
head-group, causal attention, and a partial projection; the host sums
the two head-group partials per batch and adds b_proj.

Device math (per core, all matmuls fp32r):
  qT[c,t]  = sum_k Wq_aug[k,c] xT_aug[k,t]   (bias folded via ones row)
  kT, v    similarly (v in natural [token, col] layout, with an
            interleaved ones column per head for the softmax denominator)
  scoresT[j,i] = kT.T q / 8   per head, exp'd on ScalarE (no max
            subtraction needed: scores ~ N(0,1)), causal-masked by
            multiplying precomputed 0/1 masks on the diagonal blocks
  yT_un[d,i], Z[i] = PV matmul with ones column (m=65)
  yT = yT_un * (1/Z) broadcast (gpsimd partition_broadcast), off the
            PE critical path so PSUM slots recycle fast (keeps HAM warm)
  outT partial = yT.T @ Wp   (natural [token, col] output)
"""

import sys
import os

for _p in ("/opt/trn_rl_repo", "/root/.axon_site/_ro/trn_rl_repo"):
    if os.path.isdir(_p) and _p not in sys.path:
        sys.path.insert(0, _p)

import numpy as np
import concourse.bass as bass  # noqa: F401
import concourse.mybir as mybir
import concourse.tile as tile
from concourse import bacc, bass_utils

F32 = mybir.dt.float32
F32R = mybir.dt.float32r
ActF = mybir.ActivationFunctionType

B, S, D, H = 4, 2048, 1024, 16
NH = 8          # heads per core
HPAIRS = NH // 2
KT = D // 128   # 8 k-tiles over D
N_CORES = 8

_nc_cache = {}


def build_nc(S_tok=S, n_cores=N_CORES):
    key = (S_tok, n_cores)
    if key in _nc_cache:
        return _nc_cache[key]
    IC = S_tok // 512      # query chunks
    NT = S_tok // 128      # token tiles
    nc = bacc.Bacc("TRN2", target_bir_lowering=False, debug=False,
                   num_devices=n_cores)
    xT = nc.dram_tensor("xT", [D, S_tok], F32, kind="ExternalInput").ap()
    Wq = nc.dram_tensor("Wq", [D + 1, 512], F32, kind="ExternalInput").ap()
    Wk = nc.dram_tensor("Wk", [D + 1, 512], F32, kind="ExternalInput").ap()
    Wv = nc.dram_tensor("Wv", [D + 1, 512], F32, kind="ExternalInput").ap()
    Wp = nc.dram_tensor("Wp", [512, D], F32, kind="ExternalInput").ap()
    out = nc.dram_tensor("out", [S_tok, D], F32, kind="ExternalOutput").ap()

    with tile.TileContext(nc) as tc:
        with tc.tile_pool(name="persist", bufs=1) as pp:
            # resident xT (f32r views of the f32 dram rows); DMAs are
            # issued inside the V-pass scope, interleaved with the Wv
            # loads and column-chunked so the first matmuls start early
            xtr = [pp.tile([128, S_tok], F32R, name=f"xtr{k}")
                   for k in range(KT)]
            # constant ones row: rhs/lhsT for the folded-bias K=1 matmuls
            ones_f32 = pp.tile([1, 512], F32, name="ones_f32")
            nc.gpsimd.memset(ones_f32, 1.0)
            ones_row = pp.tile([1, 512], F32R, name="ones_row")
            nc.vector.tensor_copy(ones_row, ones_f32)

            # v in natural layout, 65-stride per head (64 v cols + ones col)
            v_sb = [pp.tile([128, 8 * 65], F32R, name=f"vsb{t}")
                    for t in range(NT)]
            # yT accumulation per head pair [local d, tokens]
            yT = [pp.tile([128, S_tok], F32R, name=f"ytr{h}")
                  for h in range(HPAIRS)]
            # causal masks for the 4 diagonal offsets r = 128*t:
            # mask[p, y] = 1 if y >= p + r else 0
            masks = []
            BF16 = mybir.dt.bfloat16
            for t in range(4):
                wm = 128 * (t + 1)
                m = pp.tile([128, wm], BF16, name=f"mask{t}")
                nc.gpsimd.memset(m, 1.0)
                nc.gpsimd.affine_select(
                    out=m, in_=m, compare_op=mybir.AluOpType.is_ge,
                    fill=0.0, base=-128 * t, pattern=[[1, wm]],
                    channel_multiplier=-1)
                masks.append(m)
            ones8 = pp.tile([128, 8, 1], F32, name="ones8")
            nc.gpsimd.memset(ones8, 1.0)

            # ---- V pass ----
            with tc.tile_pool(name="wv", bufs=1) as wvp, \
                 tc.tile_pool(name="ps1", bufs=4, space="PSUM") as ps1:
                wv = [wvp.tile([128, 512], F32R, name=f"wv{k}")
                      for k in range(KT)]
                wv9 = wvp.tile([1, 512], F32R, name="wv9")
                nc.sync.dma_start(wv9, Wv[D:D + 1, :].bitcast(F32R))
                for k in range(KT):
                    nc.sync.dma_start(
                        wv[k], Wv[k * 128:(k + 1) * 128, :].bitcast(F32R))
                    nc.sync.dma_start(
                        xtr[k][:, 0:512],
                        xT[k * 128:(k + 1) * 128, 0:512].bitcast(F32R))
                for c in range(1, S_tok // 512):
                    cs = slice(c * 512, (c + 1) * 512)
                    for k in range(KT):
                        nc.sync.dma_start(
                            xtr[k][:, cs],
                            xT[k * 128:(k + 1) * 128, cs].bitcast(F32R))
                for g in range(NT // 4):
                    psvs = [ps1.tile([128, 512], F32, tag="psv", name="psv")
                            for _ in range(4)]
                    # k-outer so each wv[k] weight load serves 4 matmuls
                    for k in range(KT):
                        for j in range(4):
                            t = g * 4 + j
                            nc.tensor.matmul(
                                psvs[j], xtr[k][:, t * 128:(t + 1) * 128],
                                wv[k], start=(k == 0), stop=False)
                    for j in range(4):
                        nc.tensor.matmul(psvs[j], ones_row[0:1, 0:128],
                                         wv9, start=False, stop=True)
                    for j in range(4):
                        t = g * 4 + j
                        vv = v_sb[t].rearrange("p (h c) -> p h c", c=65)
                        nc.scalar.activation(
                            vv[:, :, 0:64],
                            psvs[j].rearrange("p (h c) -> p h c", c=64),
                            ActF.Copy)
                        nc.vector.tensor_copy(vv[:, :, 64:65], ones8)

            # ---- per head-pair: q/k then attention ----
            with tc.tile_pool(name="hsb", bufs=1) as hsb, \
                 tc.tile_pool(name="pss", bufs=2, space="PSUM") as pssp, \
                 tc.tile_pool(name="psy", bufs=4, space="PSUM") as psyp:
                def fetch_w(hp):
                    wq, wk = [], []
                    for k in range(KT):
                        tq = hsb.tile([128, 128], F32R, tag=f"wq{k}",
                                      name="wq")
                        nc.sync.dma_start(
                            tq, Wq[k * 128:(k + 1) * 128,
                                   hp * 128:(hp + 1) * 128].bitcast(F32R))
                        wq.append(tq)
                        tk = hsb.tile([128, 128], F32R, tag=f"wk{k}",
                                      name="wk")
                        nc.sync.dma_start(
                            tk, Wk[k * 128:(k + 1) * 128,
                                   hp * 128:(hp + 1) * 128].bitcast(F32R))
                        wk.append(tk)
                    wq9 = hsb.tile([1, 128], F32R, tag="wq9", name="wq9")
                    nc.sync.dma_start(
                        wq9, Wq[D:D + 1,
                                hp * 128:(hp + 1) * 128].bitcast(F32R))
                    wk9 = hsb.tile([1, 128], F32R, tag="wk9", name="wk9")
                    nc.sync.dma_start(
                        wk9, Wk[D:D + 1,
                                hp * 128:(hp + 1) * 128].bitcast(F32R))
                    return wq, wk, wq9, wk9

                wnext = fetch_w(0)
                for hp in range(HPAIRS):
                    wq, wk, wq9, wk9 = wnext

                    qt = hsb.tile([128, S_tok], F32R, tag="qt", name="qt")
                    kt_t = hsb.tile([128, S_tok], F32R, tag="kt", name="kt")
                    for dst, w, w9 in ((qt, wq, wq9), (kt_t, wk, wk9)):
                        for half in range(S_tok // 1024):
                            psq = pssp.tile([128, 1024], F32, tag="pss",
                                            name="psq")
                            # k-outer: each w[k] weight load serves 2 matmuls
                            for k in range(KT):
                                for sub in range(2):
                                    ch = half * 2 + sub
                                    o = sub * 512
                                    nc.tensor.matmul(
                                        psq[:, o:o + 512], w[k],
                                        xtr[k][:, ch * 512:(ch + 1) * 512],
                                        start=(k == 0), stop=False)
                            for sub in range(2):
                                o = sub * 512
                                nc.tensor.matmul(
                                    psq[:, o:o + 512], w9, ones_row,
                                    start=False, stop=True)
                            nc.scalar.activation(
                                dst[:, half * 1024:(half + 1) * 1024], psq,
                                ActF.Copy)

                    # prefetch next head-pair's weights now, so their
                    # DMAs aren't queued behind this pair's evacuations
                    if hp + 1 < HPAIRS:
                        wnext = fetch_w(hp + 1)

                    # attention for the 2 heads of this pair.
                    # PV results are staged UNNORMALIZED (plus the Z row);
                    # the divide by Z runs on SBUF tiles only, off the PE
                    # critical path, so PSUM slots free immediately.
                    for icp in range(IC // 2):
                        ics = [2 * icp, 2 * icp + 1]
                        psys = {}
                        for ic in ics:
                            psys[(ic, 0)] = psyp.tile([65, 512], F32,
                                                      tag="psy", name="psyA")
                            psys[(ic, 1)] = psyp.tile([65, 512], F32,
                                                      tag="psy", name="psyB")
                        # j-tiles shared across the ic pair: each kT / v
                        # weight load serves both i-chunks
                        for jt in range(4 * ics[1] + 4):
                            ets = {}
                            for ic in ics:
                                if jt >= 4 * ic + 4:
                                    continue
                                pss = pssp.tile([128, 1024], F32, tag="pss",
                                                name="pss")
                                nc.tensor.matmul(
                                    pss[:, 0:512],
                                    kt_t[0:64, jt * 128:(jt + 1) * 128],
                                    qt[0:64, ic * 512:(ic + 1) * 512],
                                    start=True, stop=True,
                                    tile_position=(0, 0))
                                nc.tensor.matmul(
                                    pss[:, 512:1024],
                                    kt_t[64:128, jt * 128:(jt + 1) * 128],
                                    qt[64:128, ic * 512:(ic + 1) * 512],
                                    start=True, stop=True,
                                    tile_position=(64, 0))
                                et = hsb.tile([128, 1024], F32R, tag="et",
                                              bufs=3, name="et")
                                nc.scalar.activation(et, pss, ActF.Exp,
                                                     scale=0.125)
                                tdx = jt - 4 * ic
                                if tdx >= 0:
                                    w_ = 128 * (tdx + 1)
                                    nc.vector.tensor_mul(
                                        et[:, 0:w_], et[:, 0:w_], masks[tdx])
                                    nc.vector.tensor_mul(
                                        et[:, 512:512 + w_],
                                        et[:, 512:512 + w_], masks[tdx])
                                ets[ic] = et
                            for head in range(2):
                                vsl = v_sb[jt][:, (2 * hp + head) * 65:
                                               (2 * hp + head) * 65 + 65]
                                for ic in ics:
                                    if jt >= 4 * ic + 4:
                                        continue
                                    nc.tensor.matmul(
                                        psys[(ic, head)], vsl,
                                        ets[ic][:, head * 512:
                                                (head + 1) * 512],
                                        start=(jt == 0),
                                        stop=(jt == 4 * ic + 3))
                        for ic in ics:
                            # fast PSUM evacuation + SBUF-only normalize
                            sl = slice(ic * 512, (ic + 1) * 512)
                            zc = hsb.tile([1, 1024], F32, tag="zc", bufs=1,
                                          name="zc")
                            for head in range(2):
                                t65 = hsb.tile([65, 512], F32R, tag="t65",
                                               bufs=2, name="t65")
                                nc.vector.tensor_copy(t65, psys[(ic, head)])
                                nc.sync.dma_start(
                                    yT[hp][head * 64:(head + 1) * 64, sl],
                                    t65[0:64, :])
                                nc.sync.dma_start(
                                    zc[0:1, head * 512:(head + 1) * 512]
                                    .bitcast(F32R), t65[64:65, :])
                            # reciprocal at full lane width: scatter the
                            # 1024 Z values over 128 partitions, recip,
                            # gather back (1-lane recip costs 6.5us)
                            zs = hsb.tile([128, 8], F32, tag="zs", bufs=2,
                                          name="zs")
                            nc.sync.dma_start(zs, zc)
                            nc.vector.reciprocal(zs, zs)
                            nc.sync.dma_start(zc, zs)
                            bcf = hsb.tile([128, 512], F32, tag="bcf",
                                           bufs=2, name="bcf")
                            nc.gpsimd.partition_broadcast(
                                bcf, zc[0:1, 512:1024])
                            nc.gpsimd.partition_broadcast(
                                bcf[0:64, :], zc[0:1, 0:512])
                            nc.vector.tensor_mul(yT[hp][:, sl],
                                                 yT[hp][:, sl], bcf)

                # ---- projection (inside the same pools: pso tiles come
                # from the pss pool so no PSUM pool boundary blocks it) ----
                with tc.tile_pool(name="wpp", bufs=1) as wpp, \
                     tc.tile_pool(name="osb", bufs=1) as osb:
                    wp = []
                    for k in range(HPAIRS):
                        t = wpp.tile([128, D], F32R, name=f"wp{k}")
                        nc.sync.dma_start(
                            t, Wp[k * 128:(k + 1) * 128, :].bitcast(F32R))
                        wp.append(t)
                    for tt in range(NT):
                        pso = pssp.tile([128, 1024], F32, tag="pss",
                                        name="pso")
                        # k-outer: each yT weight load serves both col chunks
                        for k in range(HPAIRS):
                            for nch in range(2):
                                nc.tensor.matmul(
                                    pso[:, nch * 512:(nch + 1) * 512],
                                    yT[k][:, tt * 128:(tt + 1) * 128],
                                    wp[k][:, nch * 512:(nch + 1) * 512],
                                    start=(k == 0), stop=(k == HPAIRS - 1))
                        for nch in range(2):
                            ot = osb.tile([128, 512], F32, tag="ot",
                                          bufs=3, name="ot")
                            nc.scalar.activation(
                                ot, pso[:, nch * 512:(nch + 1) * 512],
                                ActF.Copy)
                            nc.sync.dma_start(
                                out[tt * 128:(tt + 1) * 128,
                                    nch * 512:(nch + 1) * 512], ot)
    nc.finalize()
    _nc_cache[key] = nc
    return nc


def make_in_maps(x, W_attn, b_attn, W_proj):
    """Build per-core input dicts from full inputs."""
    Bx, Sx, Dx = x.shape
    in_maps = []
    for c in range(N_CORES):
        b = c // 2
        g = c % 2
        cs = slice(g * 512, (g + 1) * 512)
        xT_aug = np.ascontiguousarray(x[b].T)
        wq = np.concatenate([W_attn[:, 0:D][:, cs],
                             b_attn[0:D][cs][None, :]], axis=0)
        wk = np.concatenate([W_attn[:, D:2 * D][:, cs],
                             b_attn[D:2 * D][cs][None, :]], axis=0)
        wv = np.concatenate([W_attn[:, 2 * D:3 * D][:, cs],
                             b_attn[2 * D:3 * D][cs][None, :]], axis=0)
        wp = np.ascontiguousarray(W_proj[cs, :])
        in_maps.append({
            "xT": np.ascontiguousarray(xT_aug, dtype=np.float32),
            "Wq": np.ascontiguousarray(wq, dtype=np.float32),
            "Wk": np.ascontiguousarray(wk, dtype=np.float32),
            "Wv": np.ascontiguousarray(wv, dtype=np.float32),
            "Wp": wp.astype(np.float32),
        })
    return in_maps


def kernel(x, W_attn, b_attn, W_proj, b_proj, trace=False):
    x = np.asarray(x, dtype=np.float32)
    W_attn = np.asarray(W_attn, dtype=np.float32)
    b_attn = np.asarray(b_attn, dtype=np.float32)
    W_proj = np.asarray(W_proj, dtype=np.float32)
    b_proj = np.asarray(b_proj, dtype=np.float32)
    nc = build_nc(x.shape[1], N_CORES)
    in_maps = make_in_maps(x, W_attn, b_attn, W_proj)
    res = bass_utils.run_bass_kernel_spmd(
        nc, in_maps, core_ids=list(range(N_CORES)), trace=trace)
    Bx, Sx, Dx = x.shape
    outp = np.empty((Bx, Sx, Dx), dtype=np.float32)
    for b in range(Bx):
        outp[b] = (res.results[2 * b]["out"] + res.results[2 * b + 1]["out"]
                   + b_proj[None, :])
    if trace:
        return outp, res
    return outp


# revision 29
# speedup vs baseline: 1.3965x; 1.0466x over previous
"""Causal self-attention kernel for 8 trn2 NeuronCores.

Sharding: 4 batches x 2 head-groups (8 heads each). Core c handles
batch c//2, heads (c%2)*8 .. (c%2)*8+8. Each core computes qkv for its
head-group, causal attention, and a partial projection; the host sums
the two head-group partials per batch and adds b_proj.

Device math (per core, all matmuls fp32r):
  qT[c,t]  = sum_k Wq_aug[k,c] xT_aug[k,t]   (bias folded via ones row)
  kT, v    similarly (v in natural [token, col] layout, with an
            interleaved ones column per head for the softmax denominator)
  scoresT[j,i] = kT.T q / 8   per head, exp'd on ScalarE (no max
            subtraction needed: scores ~ N(0,1)), causal-masked by
            multiplying precomputed 0/1 masks on the diagonal blocks
  yT_un[d,i], Z[i] = PV matmul with ones column (m=65)
  yT = yT_un * (1/Z) broadcast (gpsimd partition_broadcast), off the
            PE critical path so PSUM slots recycle fast (keeps HAM warm)
  outT partial = yT.T @ Wp   (natural [token, col] output)
"""

import sys
import os

for _p in ("/opt/trn_rl_repo", "/root/.axon_site/_ro/trn_rl_repo"):
    if os.path.isdir(_p) and _p not in sys.path:
        sys.path.insert(0, _p)

import numpy as np
import concourse.bass as bass  # noqa: F401
import concourse.mybir as mybir
import concourse.tile as tile
from concourse import bacc, bass_utils

F32 = mybir.dt.float32
F32R = mybir.dt.float32r
ActF = mybir.ActivationFunctionType

B, S, D, H = 4, 2048, 1024, 16
NH = 8          # heads per core
HPAIRS = NH // 2
KT = D // 128   # 8 k-tiles over D
N_CORES = 8

_nc_cache = {}


def build_nc(S_tok=S, n_cores=N_CORES):
    key = (S_tok, n_cores)
    if key in _nc_cache:
        return _nc_cache[key]
    IC = S_tok // 512      # query chunks
    NT = S_tok // 128      # token tiles
    nc = bacc.Bacc("TRN2", target_bir_lowering=False, debug=False,
                   num_devices=n_cores)
    xT = nc.dram_tensor("xT", [D, S_tok], F32, kind="ExternalInput").ap()
    Wq = nc.dram_tensor("Wq", [D + 1, 512], F32, kind="ExternalInput").ap()
    Wk = nc.dram_tensor("Wk", [D + 1, 512], F32, kind="ExternalInput").ap()
    Wv = nc.dram_tensor("Wv", [D + 1, 512], F32, kind="ExternalInput").ap()
    Wp = nc.dram_tensor("Wp", [512, D], F32, kind="ExternalInput").ap()
    out = nc.dram_tensor("out", [S_tok, D], F32, kind="ExternalOutput").ap()

    with tile.TileContext(nc) as tc:
        with tc.tile_pool(name="persist", bufs=1) as pp:
            # resident xT (f32r views of the f32 dram rows); DMAs are
            # issued inside the V-pass scope, interleaved with the Wv
            # loads and column-chunked so the first matmuls start early
            xtr = [pp.tile([128, S_tok], F32R, name=f"xtr{k}")
                   for k in range(KT)]
            # constant ones row: rhs/lhsT for the folded-bias K=1 matmuls
            ones_f32 = pp.tile([1, 512], F32, name="ones_f32")
            nc.gpsimd.memset(ones_f32, 1.0)
            ones_row = pp.tile([1, 512], F32R, name="ones_row")
            nc.vector.tensor_copy(ones_row, ones_f32)

            # v in natural layout, 65-stride per head (64 v cols + ones col)
            v_sb = [pp.tile([128, 8 * 65], F32R, name=f"vsb{t}")
                    for t in range(NT)]
            # yT accumulation per head pair [local d, tokens]
            yT = [pp.tile([128, S_tok], F32R, name=f"ytr{h}")
                  for h in range(HPAIRS)]
            # causal masks for the 4 diagonal offsets r = 128*t:
            # mask[p, y] = 1 if y >= p + r else 0
            masks = []
            BF16 = mybir.dt.bfloat16
            for t in range(4):
                wm = 128 * (t + 1)
                m = pp.tile([128, wm], BF16, name=f"mask{t}")
                nc.gpsimd.memset(m, 1.0)
                nc.gpsimd.affine_select(
                    out=m, in_=m, compare_op=mybir.AluOpType.is_ge,
                    fill=0.0, base=-128 * t, pattern=[[1, wm]],
                    channel_multiplier=-1)
                masks.append(m)
            ones8 = pp.tile([128, 8, 1], F32, name="ones8")
            nc.gpsimd.memset(ones8, 1.0)

            # ---- V pass ----
            with tc.tile_pool(name="wv", bufs=1) as wvp, \
                 tc.tile_pool(name="ps1", bufs=4, space="PSUM") as ps1:
                wv = [wvp.tile([128, 512], F32R, name=f"wv{k}")
                      for k in range(KT)]
                wv9 = wvp.tile([1, 512], F32R, name="wv9")
                nc.sync.dma_start(wv9, Wv[D:D + 1, :].bitcast(F32R))
                for k in range(KT):
                    nc.sync.dma_start(
                        wv[k], Wv[k * 128:(k + 1) * 128, :].bitcast(F32R))
                    nc.sync.dma_start(
                        xtr[k][:, 0:512],
                        xT[k * 128:(k + 1) * 128, 0:512].bitcast(F32R))
                for c in range(1, S_tok // 512):
                    cs = slice(c * 512, (c + 1) * 512)
                    for k in range(KT):
                        nc.sync.dma_start(
                            xtr[k][:, cs],
                            xT[k * 128:(k + 1) * 128, cs].bitcast(F32R))
                for g in range(NT // 4):
                    psvs = [ps1.tile([128, 512], F32, tag="psv", name="psv")
                            for _ in range(4)]
                    # k-outer so each wv[k] weight load serves 4 matmuls
                    for k in range(KT):
                        for j in range(4):
                            t = g * 4 + j
                            nc.tensor.matmul(
                                psvs[j], xtr[k][:, t * 128:(t + 1) * 128],
                                wv[k], start=(k == 0), stop=False)
                    for j in range(4):
                        nc.tensor.matmul(psvs[j], ones_row[0:1, 0:128],
                                         wv9, start=False, stop=True)
                    for j in range(4):
                        t = g * 4 + j
                        vv = v_sb[t].rearrange("p (h c) -> p h c", c=65)
                        nc.scalar.activation(
                            vv[:, :, 0:64],
                            psvs[j].rearrange("p (h c) -> p h c", c=64),
                            ActF.Copy)
                        nc.vector.tensor_copy(vv[:, :, 64:65], ones8)

            # ---- per head-pair: q/k then attention ----
            with tc.tile_pool(name="hsb", bufs=1) as hsb, \
                 tc.tile_pool(name="pss", bufs=2, space="PSUM") as pssp, \
                 tc.tile_pool(name="psy", bufs=4, space="PSUM") as psyp:
                def fetch_w(hp):
                    wq, wk = [], []
                    for k in range(KT):
                        tq = hsb.tile([128, 128], F32R, tag=f"wq{k}",
                                      name="wq")
                        nc.sync.dma_start(
                            tq, Wq[k * 128:(k + 1) * 128,
                                   hp * 128:(hp + 1) * 128].bitcast(F32R))
                        wq.append(tq)
                        tk = hsb.tile([128, 128], F32R, tag=f"wk{k}",
                                      name="wk")
                        nc.sync.dma_start(
                            tk, Wk[k * 128:(k + 1) * 128,
                                   hp * 128:(hp + 1) * 128].bitcast(F32R))
                        wk.append(tk)
                    wq9 = hsb.tile([128, 1], F32, tag="wq9", name="wq9")
                    nc.sync.dma_start(
                        wq9, Wq[D:D + 1, hp * 128:(hp + 1) * 128])
                    wk9 = hsb.tile([128, 1], F32, tag="wk9", name="wk9")
                    nc.sync.dma_start(
                        wk9, Wk[D:D + 1, hp * 128:(hp + 1) * 128])
                    return wq, wk, wq9, wk9

                wnext = fetch_w(0)
                for hp in range(HPAIRS):
                    wq, wk, wq9, wk9 = wnext

                    qt = hsb.tile([128, S_tok], F32R, tag="qt", name="qt")
                    kt_t = hsb.tile([128, S_tok], F32R, tag="kt", name="kt")
                    for dst, w, w9 in ((qt, wq, wq9), (kt_t, wk, wk9)):
                        for half in range(S_tok // 1024):
                            psq = pssp.tile([128, 1024], F32, tag="pss",
                                            name="psq")
                            # k-outer: each w[k] weight load serves 2 matmuls
                            for k in range(KT):
                                for sub in range(2):
                                    ch = half * 2 + sub
                                    o = sub * 512
                                    nc.tensor.matmul(
                                        psq[:, o:o + 512], w[k],
                                        xtr[k][:, ch * 512:(ch + 1) * 512],
                                        start=(k == 0), stop=(k == KT - 1))
                            # bias folded into the evacuation copy
                            nc.scalar.activation(
                                dst[:, half * 1024:(half + 1) * 1024], psq,
                                ActF.Identity, bias=w9)

                    # prefetch next head-pair's weights now, so their
                    # DMAs aren't queued behind this pair's evacuations
                    if hp + 1 < HPAIRS:
                        wnext = fetch_w(hp + 1)

                    # attention for the 2 heads of this pair.
                    # PV results are staged UNNORMALIZED (plus the Z row);
                    # the divide by Z runs on SBUF tiles only, off the PE
                    # critical path, so PSUM slots free immediately.
                    for icp in range(IC // 2):
                        ics = [2 * icp, 2 * icp + 1]
                        psys = {}
                        for ic in ics:
                            psys[(ic, 0)] = psyp.tile([65, 512], F32,
                                                      tag="psy", name="psyA")
                            psys[(ic, 1)] = psyp.tile([65, 512], F32,
                                                      tag="psy", name="psyB")
                        # j-tiles shared across the ic pair: each kT / v
                        # weight load serves both i-chunks
                        for jt in range(4 * ics[1] + 4):
                            ets = {}
                            for ic in ics:
                                if jt >= 4 * ic + 4:
                                    continue
                                pss = pssp.tile([128, 1024], F32, tag="pss",
                                                name="pss")
                                nc.tensor.matmul(
                                    pss[:, 0:512],
                                    kt_t[0:64, jt * 128:(jt + 1) * 128],
                                    qt[0:64, ic * 512:(ic + 1) * 512],
                                    start=True, stop=True,
                                    tile_position=(0, 0))
                                nc.tensor.matmul(
                                    pss[:, 512:1024],
                                    kt_t[64:128, jt * 128:(jt + 1) * 128],
                                    qt[64:128, ic * 512:(ic + 1) * 512],
                                    start=True, stop=True,
                                    tile_position=(64, 0))
                                et = hsb.tile([128, 1024], F32R, tag="et",
                                              bufs=3, name="et")
                                nc.scalar.activation(et, pss, ActF.Exp,
                                                     scale=0.125)
                                tdx = jt - 4 * ic
                                if tdx >= 0:
                                    w_ = 128 * (tdx + 1)
                                    nc.vector.tensor_mul(
                                        et[:, 0:w_], et[:, 0:w_], masks[tdx])
                                    nc.vector.tensor_mul(
                                        et[:, 512:512 + w_],
                                        et[:, 512:512 + w_], masks[tdx])
                                ets[ic] = et
                            for head in range(2):
                                vsl = v_sb[jt][:, (2 * hp + head) * 65:
                                               (2 * hp + head) * 65 + 65]
                                for ic in ics:
                                    if jt >= 4 * ic + 4:
                                        continue
                                    nc.tensor.matmul(
                                        psys[(ic, head)], vsl,
                                        ets[ic][:, head * 512:
                                                (head + 1) * 512],
                                        start=(jt == 0),
                                        stop=(jt == 4 * ic + 3))
                        for ic in ics:
                            # fast PSUM evacuation + SBUF-only normalize
                            sl = slice(ic * 512, (ic + 1) * 512)
                            zc = hsb.tile([1, 1024], F32, tag="zc", bufs=1,
                                          name="zc")
                            for head in range(2):
                                t65 = hsb.tile([65, 512], F32R, tag="t65",
                                               bufs=2, name="t65")
                                nc.vector.tensor_copy(t65, psys[(ic, head)])
                                nc.sync.dma_start(
                                    yT[hp][head * 64:(head + 1) * 64, sl],
                                    t65[0:64, :])
                                nc.sync.dma_start(
                                    zc[0:1, head * 512:(head + 1) * 512]
                                    .bitcast(F32R), t65[64:65, :])
                            # reciprocal at full lane width: scatter the
                            # 1024 Z values over 128 partitions, recip,
                            # gather back (1-lane recip costs 6.5us)
                            zs = hsb.tile([128, 8], F32, tag="zs", bufs=2,
                                          name="zs")
                            nc.sync.dma_start(zs, zc)
                            nc.vector.reciprocal(zs, zs)
                            nc.sync.dma_start(zc, zs)
                            bcf = hsb.tile([128, 512], F32, tag="bcf",
                                           bufs=2, name="bcf")
                            nc.gpsimd.partition_broadcast(
                                bcf, zc[0:1, 512:1024])
                            nc.gpsimd.partition_broadcast(
                                bcf[0:64, :], zc[0:1, 0:512])
                            nc.vector.tensor_mul(yT[hp][:, sl],
                                                 yT[hp][:, sl], bcf)

                # ---- projection (inside the same pools: pso tiles come
                # from the pss pool so no PSUM pool boundary blocks it) ----
                with tc.tile_pool(name="wpp", bufs=1) as wpp, \
                     tc.tile_pool(name="osb", bufs=1) as osb:
                    wp = []
                    for k in range(HPAIRS):
                        t = wpp.tile([128, D], F32R, name=f"wp{k}")
                        nc.sync.dma_start(
                            t, Wp[k * 128:(k + 1) * 128, :].bitcast(F32R))
                        wp.append(t)
                    for tt in range(NT):
                        pso = pssp.tile([128, 1024], F32, tag="pss",
                                        name="pso")
                        # k-outer: each yT weight load serves both col chunks
                        for k in range(HPAIRS):
                            for nch in range(2):
                                nc.tensor.matmul(
                                    pso[:, nch * 512:(nch + 1) * 512],
                                    yT[k][:, tt * 128:(tt + 1) * 128],
                                    wp[k][:, nch * 512:(nch + 1) * 512],
                                    start=(k == 0), stop=(k == HPAIRS - 1))
                        for nch in range(2):
                            ot = osb.tile([128, 512], F32, tag="ot",
                                          bufs=3, name="ot")
                            nc.scalar.activation(
                                ot, pso[:, nch * 512:(nch + 1) * 512],
                                ActF.Copy)
                            nc.sync.dma_start(
                                out[tt * 128:(tt + 1) * 128,
                                    nch * 512:(nch + 1) * 512], ot)
    nc.finalize()
    _nc_cache[key] = nc
    return nc


def make_in_maps(x, W_attn, b_attn, W_proj):
    """Build per-core input dicts from full inputs."""
    Bx, Sx, Dx = x.shape
    in_maps = []
    for c in range(N_CORES):
        b = c // 2
        g = c % 2
        cs = slice(g * 512, (g + 1) * 512)
        xT_aug = np.ascontiguousarray(x[b].T)
        wq = np.concatenate([W_attn[:, 0:D][:, cs],
                             b_attn[0:D][cs][None, :]], axis=0)
        wk = np.concatenate([W_attn[:, D:2 * D][:, cs],
                             b_attn[D:2 * D][cs][None, :]], axis=0)
        wv = np.concatenate([W_attn[:, 2 * D:3 * D][:, cs],
                             b_attn[2 * D:3 * D][cs][None, :]], axis=0)
        wp = np.ascontiguousarray(W_proj[cs, :])
        in_maps.append({
            "xT": np.ascontiguousarray(xT_aug, dtype=np.float32),
            "Wq": np.ascontiguousarray(wq, dtype=np.float32),
            "Wk": np.ascontiguousarray(wk, dtype=np.float32),
            "Wv": np.ascontiguousarray(wv, dtype=np.float32),
            "Wp": wp.astype(np.float32),
        })
    return in_maps


def kernel(x, W_attn, b_attn, W_proj, b_proj, trace=False):
    x = np.asarray(x, dtype=np.float32)
    W_attn = np.asarray(W_attn, dtype=np.float32)
    b_attn = np.asarray(b_attn, dtype=np.float32)
    W_proj = np.asarray(W_proj, dtype=np.float32)
    b_proj = np.asarray(b_proj, dtype=np.float32)
    nc = build_nc(x.shape[1], N_CORES)
    in_maps = make_in_maps(x, W_attn, b_attn, W_proj)
    res = bass_utils.run_bass_kernel_spmd(
        nc, in_maps, core_ids=list(range(N_CORES)), trace=trace)
    Bx, Sx, Dx = x.shape
    outp = np.empty((Bx, Sx, Dx), dtype=np.float32)
    for b in range(Bx):
        outp[b] = (res.results[2 * b]["out"] + res.results[2 * b + 1]["out"]
                   + b_proj[None, :])
    if trace:
        return outp, res
    return outp
